# revision 14
# baseline (speedup 1.0000x reference)
"""Gumbel-Sinkhorn kernel for Trainium2 (8 NeuronCores, data-parallel over batch).

Algorithm: the reference runs 60 Sinkhorn normalization sweeps over P0 =
softmax((logits + 0.01*gumbel)/3).  Because row/col normalization preserves
the form P_t = diag(r) @ K @ diag(c) (K = exp(y), unnormalized), each sweep
is equivalent to the scaling-vector updates
    c <- 1 / (K^T r),   r <- 1 / (K c)
starting from r0 = 1/rowsum(K) (which absorbs the softmax denominator).
The iteration contracts ~1e-3 per sweep for this temperature, so T=4 sweeps
match the 60-sweep fp32 reference to the fp32 noise floor (~2e-6 rel).

Per core: 32 matrices of [512,512] fp32.  Each matrix:
  - DMA logits,u; compute K = exp((logits - 0.01*ln(-ln u))/3) with fused
    per-row-block accumulated rowsums (ACT engine), r0 = 1/rowsum (DVE).
  - K^T built via 16 PE-transposes (needed so both matvec directions are
    partition-contractions).
  - T sweeps: each matvec = 4 accumulating matmuls (vector-as-weights,
    rhs streams the matrix), output [1,512] in PSUM; copy to SBUF; 4 tiny
    "conversion" matmuls flip [1,512] -> [128,4] partition layout for the
    next step's weights; reciprocal lands there (128 lanes).
  - final P = (K * r) * bcast(c): c broadcast across partitions via
    outer-product matmuls, reciprocal on [128,512], fused scalar_tensor_tensor.
"""

import numpy as np

N_CORES = 8
B_FULL = 256
BM = B_FULL // N_CORES  # 32 matrices per core
N = 512
NB = N // 128  # 4 row/col blocks
T_SWEEPS = 4
GAMMA = 0.01
TEMP = 3.0

_nc_cache = {}


def _build(bm=BM, t_sweeps=T_SWEEPS, reps=1):
    import concourse.bacc as bacc
    import concourse.mybir as mybir
    from concourse.tile import TileContext
    from concourse.masks import make_identity

    f32 = mybir.dt.float32
    AF = mybir.ActivationFunctionType
    ALU = mybir.AluOpType

    # Bacc (not plain Bass): its compile pipeline runs
    # generate_event_semaphores, which legalizes the trn2 "at most one
    # sync-wait per instruction" constraint that our cross-engine dep
    # pattern otherwise violates.
    nc = bacc.Bacc()
    lo_h = nc.dram_tensor("logits_s", [bm, N, N], f32, kind="ExternalInput")
    u_h = nc.dram_tensor("u_s", [bm, N, N], f32, kind="ExternalOutput" if False else "ExternalInput")
    out_h = nc.dram_tensor("out_s", [bm, N, N], f32, kind="ExternalOutput")

    # DRAM views: [bm, (ib p), j] -> [bm, p, ib, j]
    lo_v = lo_h.rearrange("b (a p) j -> b p a j", p=128)
    u_v = u_h.rearrange("b (a p) j -> b p a j", p=128)
    out_v = out_h.rearrange("b (a p) j -> b p a j", p=128)

    with TileContext(nc) as tc:
        with (
            tc.tile_pool(name="consts", bufs=1) as consts,
            tc.tile_pool(name="pL", bufs=2) as pL,
            tc.tile_pool(name="pU", bufs=2) as pU,
            tc.tile_pool(name="pKT", bufs=2) as pKT,
            tc.tile_pool(name="pvec", bufs=10) as pvec,
            tc.tile_pool(name="prow", bufs=6) as prow,
            tc.tile_pool(name="ps_row", bufs=2, space="PSUM") as ps_row,
            tc.tile_pool(name="ps_col", bufs=2, space="PSUM") as ps_col,
            tc.tile_pool(name="ps_big", bufs=2, space="PSUM") as ps_big,
        ):
            ident = consts.tile([128, 128], f32)
            make_identity(nc, ident)
            ones = consts.tile([1, 128], f32)
            nc.vector.memset(ones, 1.0)

            for m in [mm for _ in range(reps) for mm in range(bm)]:
                L = pL.tile([128, NB, N], f32)   # logits -> K
                U = pU.tile([128, NB, N], f32)   # u -> -eps -> output
                KT = pKT.tile([128, NB, N], f32)

                for ib in range(NB):
                    nc.sync.dma_start(out=L[:, ib, :], in_=lo_v[m][:, ib, :])
                    nc.sync.dma_start(out=U[:, ib, :], in_=u_v[m][:, ib, :])

                # per block: U = ln(-ln(u)) (= -eps); L = L - 0.01*U;
                # K = exp(L/3) with fused rowsum
                rs = pvec.tile([128, NB], f32, tag="vec")
                for ib in range(NB):
                    nc.scalar.activation(U[:, ib, :], U[:, ib, :], AF.Ln)
                    nc.scalar.activation(U[:, ib, :], U[:, ib, :], AF.Ln, scale=-1.0)
                    nc.vector.scalar_tensor_tensor(
                        L[:, ib, :], U[:, ib, :], -GAMMA, L[:, ib, :],
                        ALU.mult, ALU.add,
                    )
                    nc.scalar.activation(
                        L[:, ib, :], L[:, ib, :], AF.Exp, scale=1.0 / TEMP,
                        accum_out=rs[:, ib : ib + 1],
                    )
                r = pvec.tile([128, NB], f32, tag="vec")
                nc.vector.reciprocal(r, rs)

                # KT[:, jb, i] = K[i, jb*128 + jlocal]
                for jb in range(NB):
                    pst = ps_big.tile([128, N], f32, tag="big")
                    for ib in range(NB):
                        nc.tensor.transpose(
                            pst[:, ib * 128 : (ib + 1) * 128],
                            L[:, ib, jb * 128 : (jb + 1) * 128],
                            ident,
                        )
                    nc.scalar.activation(KT[:, jb, :], pst, AF.Copy)

                crow_last = None
                for t in range(t_sweeps):
                    for half in range(2):
                        # half 0: u = K^T r  (contracts i; rhs = K tiles)
                        # half 1: v = K c    (contracts j; rhs = KT tiles)
                        src = L if half == 0 else KT
                        vec = r if half == 0 else c  # noqa: F821
                        psr = ps_row.tile([1, N], f32, tag="psr")
                        for kb in range(NB):
                            nc.tensor.matmul(
                                psr,
                                vec[:, kb : kb + 1],
                                src[:, kb, :],
                                start=(kb == 0),
                                stop=(kb == NB - 1),
                            )
                        row = prow.tile([1, N], f32, tag="row")
                        # copy PSUM->SBUF (alternate engines to balance load)
                        if (t + half) % 2 == 0:
                            nc.scalar.activation(row, psr, AF.Copy)
                        else:
                            nc.vector.tensor_copy(row, psr)
                        # conversion: [1,512] -> [128,4] via 4 tiny matmuls
                        psc = ps_col.tile([128, NB], f32, tag="psc")
                        for q in range(NB):
                            nc.tensor.matmul(
                                psc[:, q : q + 1],
                                row[0:1, q * 128 : (q + 1) * 128],
                                ones[0:1, 0:1],
                                start=True,
                                stop=True,
                            )
                        nv = pvec.tile([128, NB], f32, tag="vec")
                        nc.vector.reciprocal(nv, psc)
                        if half == 0:
                            c = nv
                            crow_last = row  # raw K^T r (reciprocal = c)
                        else:
                            r = nv

                # final: out = (K * r) * (1 / bcast(crow_last))
                psb = ps_big.tile([128, N], f32, tag="big")
                for jb in range(NB):
                    nc.tensor.matmul(
                        psb[:, jb * 128 : (jb + 1) * 128],
                        ones[0:1, :],
                        crow_last[0:1, jb * 128 : (jb + 1) * 128],
                        start=True,
                        stop=True,
                    )
                Bc = pU.tile([128, N], f32, tag="bc")
                nc.vector.reciprocal(Bc, psb)
                for ib in range(NB):
                    nc.vector.scalar_tensor_tensor(
                        U[:, ib, :], L[:, ib, :], r[:, ib : ib + 1], Bc,
                        ALU.mult, ALU.mult,
                    )
                    nc.sync.dma_start(out=out_v[m][:, ib, :], in_=U[:, ib, :])

    return nc


def get_nc(bm=BM, t_sweeps=T_SWEEPS, reps=1):
    key = (bm, t_sweeps, reps)
    if key not in _nc_cache:
        nc = _build(bm, t_sweeps, reps)
        nc.finalize()  # Bacc: runs wait-legalization + reg alloc passes
        _nc_cache[key] = nc
    return _nc_cache[key]


def kernel(logits: np.ndarray, u: np.ndarray, trace: bool = False):
    from concourse.bass_utils import run_bass_kernel_spmd

    logits = np.ascontiguousarray(logits, dtype=np.float32)
    u = np.ascontiguousarray(u, dtype=np.float32)
    nc = get_nc()
    in_maps = [
        {"logits_s": logits[i * BM : (i + 1) * BM], "u_s": u[i * BM : (i + 1) * BM]}
        for i in range(N_CORES)
    ]
    res = run_bass_kernel_spmd(nc, in_maps, core_ids=list(range(N_CORES)), trace=trace)
    out = np.concatenate([res.results[i]["out_s"] for i in range(N_CORES)], axis=0)
    if trace:
        return out, res
    return out
